# revision 28
# baseline (speedup 1.0000x reference)
"""Bipartite GCN stack (2 layers) on 8 Trainium2 NeuronCores.

Architecture (v2): associativity + partition-aligned aggregation.

  - A @ (H W + b) == (A @ H) W + deg*b: every sparse aggregation runs on the
    RAW table (H_src / H1' / Hs1) and the dense d x d transform is applied
    per-destination afterwards.  No pre-transformed 64MB tables, no
    redundant dense work.
  - Aggregation: destinations are degree-sorted and dealt round-robin into
    128-row tiles (tile g -> core g%8), so every destination owns one SBUF
    partition.  Each gathered "column" holds one edge per destination;
    msg accumulation is a single DVE scalar_tensor_tensor per column
    (acc += gathered * val[p]), and the degree is a free-axis reduce of the
    val matrix.  No selection-matrix matmuls at all.
  - Gathers: dma_gather with 4 SWDGE queues round-robin (the Q7 ucode runs
    on core pair `queue_num`, so 4 queues pipeline ~2.4x).  65536-row
    tables are addressed with SIGNED int16 indices against a base biased by
    +32768 rows (the ucode sign-extends; only TRAILING negative indices are
    trimmed, so the planner guarantees the last slot of every call is
    non-negative via partition-127 edge placement).
  - BN stats via PE (ones^T @ [x | x^2] accumulated across tiles), 2KB
    AllReduce, coefficients broadcast via a DRAM round-trip.
  - Tables H1' (BN'd layer-1 targets) and Hs1 (layer-1 sources) are
    produced in slot order, AllGathered in bf16, and indexed through the
    host-side slot maps.

Host-side work: integer edge planning (sort/permute/pad) only; all FP math
runs on the NeuronCores.
"""

import numpy as np

P = 128
D_FIXED = 256
EPS = 1e-5
NCORES = 8
GBT = 8          # gather batch: columns (x128 rows) per dma_gather call
NSWQ = 4         # SWDGE queues used round-robin
PE_OFFLOAD = False

N_TGT = 32768
N_SRC = 65536


# ----------------------------------------------------------------- host plan


def _rank_within_group(sorted_groups):
    """Given a sorted array of group ids, return the rank of each element
    within its group (0,1,2,... per group)."""
    n = len(sorted_groups)
    if n == 0:
        return np.zeros(0, np.int64)
    first = np.r_[True, sorted_groups[1:] != sorted_groups[:-1]]
    starts = np.where(first, np.arange(n), 0)
    np.maximum.accumulate(starts, out=starts)
    return np.arange(n) - starts


def _constrained_positions(k, ncols, col0, tc):
    """Batch-final slot positions (<k) for a partition-127 destination with
    k edges in a tile spanning global columns [col0, col0+ncols) (batches cut
    at GLOBAL column multiples of 8, plus the very last column tc-1)."""
    cuts = [q - col0 for q in range(GBT - 1, col0 + ncols, GBT)
            if q >= col0]
    last = tc - 1 - col0
    if 0 <= last < ncols and last not in cuts:
        cuts.append(last)
    return [q for q in cuts if q < k]


class _SidePlan:
    pass


def _plan_side(dst, n_dst, ncores):
    """Degree-sorted partition-aligned destination layout for one direction.

    Returns a _SidePlan with:
      part:   [n_tiles, 128] destination ids per (global tile, partition)
      ncols:  [nlt] common column count per local tile
      slot:   [n_dst] -> (core*shard + lt*128 + p) table-row of each dst
      e_core/e_lt/e_p: per-edge placement (column assigned later per pass)
    """
    sp = _SidePlan()
    counts = np.bincount(dst, minlength=n_dst)
    order = np.argsort(-counts, kind="stable")
    n_tiles = n_dst // P
    nlt = n_tiles // ncores
    part = order.reshape(n_tiles, P).copy()
    band_max = counts[order].reshape(nlt, ncores * P).max(axis=1)
    ncols = np.maximum(band_max, 1).astype(np.int64)

    sp.counts = counts
    sp.part = part
    sp.ncols = ncols
    sp.n_tiles = n_tiles
    sp.nlt = nlt
    sp.n_dst = n_dst
    return sp


def _finish_side(sp, ncores, good_masks):
    """Pick partition-127 members (trailing-trim guard) and build slot maps.

    good_masks: list of [n_dst] bool arrays, one per biased pass using this
    side's layout (destination d needs >= |constrained| good edges for EVERY
    pass).  Empty list -> no constraint.
    """
    counts, part, ncols = sp.counts, sp.part, sp.ncols
    col_off = np.concatenate([[0], np.cumsum(ncols)])
    tc = int(col_off[-1])
    if good_masks:
        # per-destination good-edge counts per pass
        for g in range(sp.n_tiles):
            lt = g // ncores
            m = int(ncols[lt])
            c0 = int(col_off[lt])
            members = part[g]
            best, best_slack = 127, None
            for j in range(P):
                t = members[j]
                k = int(counts[t])
                ncon = len(_constrained_positions(k, m, c0, tc))
                slack = min(int(gm[t]) - ncon for gm in good_masks)
                if best_slack is None or slack > best_slack:
                    best, best_slack = j, slack
                    if slack >= 2:
                        break
            assert best_slack is not None and best_slack >= 0, (
                f"tile {g}: no viable partition-127 member (slack {best_slack})"
            )
            if best != 127:
                part[g, 127], part[g, best] = part[g, best], part[g, 127]

    slot = np.empty(sp.n_dst, np.int64)
    n_tiles = sp.n_tiles
    g_idx = np.arange(n_tiles)
    core_of_g = g_idx % ncores
    lt_of_g = g_idx // ncores
    shard = sp.n_dst // ncores
    base = core_of_g * shard + lt_of_g * P
    slot[part] = base[:, None] + np.arange(P)[None, :]
    sp.slot = slot
    sp.col_off = col_off
    sp.tc = tc
    return sp


def _assign_columns(sp, dst, tbl_idx, vals, ncores, constrain_good=None):
    """Assign each edge to (core, colg, p) and build idx/val arrays.

    tbl_idx: per-edge int16 table index (may be negative for biased tables).
    constrain_good: bool[n_edges] "good" mask; if given, partition-127
    destinations get good edges placed at constrained positions.
    Returns idx16 [ncores][128, TC*8], val [ncores][128, TC] f32.
    """
    slot = sp.slot[dst]
    shard = sp.n_dst // ncores
    core = slot // shard
    rem = slot % shard
    lt = rem // P
    p = rem % P

    # rank of each edge within its destination
    eorder = np.argsort(slot, kind="stable")
    rank = np.empty(len(dst), np.int64)
    rank[eorder] = _rank_within_group(slot[eorder])

    if constrain_good is not None:
        # re-rank edges of partition-127 destinations: good edges first at
        # constrained positions
        p127 = p == 127
        if p127.any():
            sub = np.where(p127)[0]
            sub_slot = slot[sub]
            so = np.argsort(sub_slot, kind="stable")
            sub_s = sub[so]
            ss = sub_slot[so]
            starts = np.r_[0, np.where(ss[1:] != ss[:-1])[0] + 1]
            ends = np.r_[starts[1:], len(ss)]
            for a, b in zip(starts, ends):
                es = sub_s[a:b]
                k = b - a
                lt_i = (ss[a] % shard) // P
                m = int(sp.ncols[lt_i])
                cons = _constrained_positions(
                    k, m, int(sp.col_off[lt_i]), sp.tc
                )
                good = constrain_good[es]
                order_pos = np.full(k, -1, np.int64)
                gi = np.where(good)[0]
                bi = np.where(~good)[0]
                assert len(gi) >= len(cons), "p127 guard violated"
                # good edges at constrained positions, rest fill remaining
                used = set()
                for q, e in zip(cons, gi):
                    order_pos[e] = q
                    used.add(q)
                rest = [q for q in range(k) if q not in used]
                pool = [e for e in gi[len(cons):]] + list(bi)
                for q, e in zip(rest, pool):
                    order_pos[e] = q
                rank[es] = order_pos

    colg = sp.col_off[lt] + rank
    tc = sp.tc

    idx16 = []
    valo = []
    for c in range(ncores):
        sel = core == c
        lin = np.zeros((tc, P), np.int16)
        va = np.zeros((tc, P), np.float32)
        lin[colg[sel], p[sel]] = tbl_idx[sel].astype(np.int16)
        va[colg[sel], p[sel]] = vals[sel]
        # idx layout: per GLOBAL batch [b0, b0+bs): [bs*128] -> [bs*8, 16].T
        out16 = np.zeros((16, tc * 8), np.int16)
        b0 = 0
        while b0 < tc:
            bs = min(GBT, tc - b0)
            blk = lin[b0 : b0 + bs].reshape(bs * P)
            out16[:, b0 * 8 : (b0 + bs) * 8] = blk.reshape(bs * 8, 16).T
            b0 += bs
        idx16.append(np.ascontiguousarray(np.tile(out16, (8, 1))))
        valo.append(np.ascontiguousarray(va.T))
    return idx16, valo


def _make_plans(rows, cols, vals):
    """Full host plan for all three aggregation passes."""
    fwd = _plan_side(rows, N_TGT, NCORES)
    bwd = _plan_side(cols, N_SRC, NCORES)

    # bwd has no trailing-trim constraint (table indices 0..32767 >= 0)
    _finish_side(bwd, NCORES, [])

    # fwd feeds pass B (idx = cols - 32768) and pass G (idx = hsrow - 32768)
    hsrow = bwd.slot  # source id -> Hs1 table row
    goodB = np.bincount(rows[cols >= N_SRC // 2], minlength=N_TGT)
    goodG = np.bincount(rows[hsrow[cols] >= N_SRC // 2], minlength=N_TGT)
    _finish_side(fwd, NCORES, [goodB, goodG])
    h1row = fwd.slot  # target id -> H1' table row

    idxB, valB = _assign_columns(
        fwd, rows, cols - N_SRC // 2, vals, NCORES,
        constrain_good=cols >= N_SRC // 2,
    )
    gmapped = hsrow[cols]
    idxG, valG = _assign_columns(
        fwd, rows, gmapped - N_SRC // 2, vals, NCORES,
        constrain_good=gmapped >= N_SRC // 2,
    )
    idxE, valE = _assign_columns(bwd, cols, h1row[rows], vals, NCORES)

    return fwd, bwd, idxB, valB, idxE, valE, idxG, valG


# ----------------------------------------------------------------- bass build


def _install_drain_patch():
    """walrus in this env allows only ONE sem-wait per instruction; split
    extra waits onto same-engine carrier instructions."""
    import concourse.mybir as mybir
    import concourse.tile as _tile
    from concourse.vector_clock import ScopedClock

    if getattr(_tile.TileContext, "_drain_split_patched", False):
        return

    def _split_drain_and_barrier(self, tick_clock, wait_clock):
        nc = self.nc
        drain_inst = nc.sync.drain()
        wait_clock.add_sem_waits(
            drain_inst.ins, ScopedClock({None: tick_clock.global_clock})
        )
        si = drain_inst.ins.sync_info
        waits = list(si.on_wait) if si and si.on_wait else []
        if len(waits) > 1:
            si.on_wait = waits[:1]
            drain_inst.ins.sync_info = si
            for i in range(1, len(waits)):
                extra = nc.sync.drain()
                esi = extra.ins.sync_info
                upd = list(esi.on_update) if esi and esi.on_update else []
                extra.ins.sync_info = mybir.SyncInfo(
                    on_wait=[waits[i]], on_update=upd
                )
        nc.all_engine_barrier()
        assert self.sems is not None
        popped = nc._tile_sem_poison_stack.pop()
        assert popped is self._sem_poison
        nc.clear_and_free_semaphores(list(self.sems.allocated().values()))
        nc.all_engine_barrier()

    _tile.TileContext._drain_and_barrier = _split_drain_and_barrier

    _orig_add = _tile.TileContext._add_instruction

    def _add_instruction_split(self, inst):
        si = inst.sync_info
        waits = list(si.on_wait) if si and si.on_wait else []
        if len(waits) > 1 and inst.engine != mybir.EngineType.Unassigned:
            for w in waits[:-1]:
                nop = mybir.InstNoOp(
                    name=self.nc.get_next_instruction_name(), ins=[], outs=[]
                )
                nop.engine = inst.engine
                nop.sync_info = mybir.SyncInfo(on_wait=[w], on_update=[])
                _orig_add(self, nop)
            si.on_wait = waits[-1:]
            inst.sync_info = si
        _orig_add(self, inst)

    _tile.TileContext._add_instruction = _add_instruction_split
    _tile.TileContext._drain_split_patched = True


def _build_program(ncolsF, ncolsB):
    from contextlib import ExitStack

    import concourse.bass as bass
    import concourse.mybir as mybir
    import concourse.tile as tile
    from concourse import bacc
    from concourse.masks import make_identity

    _install_drain_patch()

    dt = mybir.dt
    f32 = dt.float32
    bf16 = dt.bfloat16
    i16 = dt.int16
    D = D_FIXED
    DC = D // P
    NTF = len(ncolsF)   # fwd tiles per core (32)
    NTB = len(ncolsB)   # bwd tiles per core (64)
    TCF = int(sum(ncolsF))
    TCB = int(sum(ncolsB))
    tgt_sh = NTF * P
    src_sh = NTB * P
    AluOp = mybir.AluOpType
    Act = mybir.ActivationFunctionType
    Axis = mybir.AxisListType
    rg = [list(range(NCORES))]

    nc = bacc.Bacc(
        "TRN2", target_bir_lowering=False, debug=False,
        num_devices=NCORES, num_swdge_queues=NSWQ,
    )

    dram_t = nc.dram_tensor
    # gathered with signed idx against a base biased by +32768 rows
    Hsrc = dram_t("Hsrc", [N_SRC, D], f32, kind="ExternalInput").ap()
    emb = dram_t("emb", [tgt_sh, D], f32, kind="ExternalInput").ap()
    W0 = dram_t("W0", [D, D], f32, kind="ExternalInput").ap()
    Wb = dram_t("Wb", [D, D], f32, kind="ExternalInput").ap()
    W1 = dram_t("W1", [D, D], f32, kind="ExternalInput").ap()
    b0_h = dram_t("b0", [1, D], f32, kind="ExternalInput")
    bb_h = dram_t("bb", [1, D], f32, kind="ExternalInput")
    b1_h = dram_t("b1", [1, D], f32, kind="ExternalInput")
    g1_h = dram_t("g1", [1, D], f32, kind="ExternalInput").ap()
    be1_h = dram_t("be1", [1, D], f32, kind="ExternalInput").ap()
    g2_h = dram_t("g2", [1, D], f32, kind="ExternalInput").ap()
    be2_h = dram_t("be2", [1, D], f32, kind="ExternalInput").ap()
    fe_i16 = dram_t("fe_i16", [P, TCF * 8], i16, kind="ExternalInput").ap()
    fe_val = dram_t("fe_val", [P, TCF], f32, kind="ExternalInput").ap()
    be_i16 = dram_t("be_i16", [P, TCB * 8], i16, kind="ExternalInput").ap()
    be_val = dram_t("be_val", [P, TCB], f32, kind="ExternalInput").ap()
    ge_i16 = dram_t("ge_i16", [P, TCF * 8], i16, kind="ExternalInput").ap()
    ge_val = dram_t("ge_val", [P, TCF], f32, kind="ExternalInput").ap()
    out_d = dram_t("out", [tgt_sh, D], f32, kind="ExternalOutput").ap()

    with tile.TileContext(nc) as tc, ExitStack() as ctx:
        dram = ctx.enter_context(tc.tile_pool(name="dram", bufs=1, space="DRAM"))
        X1_loc = dram.tile([tgt_sh, D], bf16)
        X1_full = dram.tile([N_TGT, D], bf16, addr_space="Shared")
        Hs1_loc = dram.tile([src_sh, D], bf16)
        Hs1_full = dram.tile([N_SRC, D], bf16, addr_space="Shared")
        st1_in = dram.tile([1, 2 * D], f32)
        st1_out = dram.tile([1, 2 * D], f32, addr_space="Shared")
        st2_in = dram.tile([1, 2 * D], f32)
        st2_out = dram.tile([1, 2 * D], f32, addr_space="Shared")
        ab1_d = dram.tile([1, 2 * D], f32)
        ab2_d = dram.tile([1, 2 * D], f32)

        # ---------------- constants ------------------------------------
        consts = ctx.enter_context(tc.tile_pool(name="consts", bufs=1))
        w0t = consts.tile([P, DC, D], f32)
        wbt = consts.tile([P, DC, D], f32)
        w1t = consts.tile([P, DC, D], f32)
        for c in range(DC):
            nc.sync.dma_start(out=w0t[:, c, :], in_=W0[c * P : (c + 1) * P, :])
            nc.sync.dma_start(out=wbt[:, c, :], in_=Wb[c * P : (c + 1) * P, :])
            nc.sync.dma_start(out=w1t[:, c, :], in_=W1[c * P : (c + 1) * P, :])
        w0b = consts.tile([P, DC, D], bf16)
        wbb = consts.tile([P, DC, D], bf16)
        w1b = consts.tile([P, DC, D], bf16)
        for c in range(DC):
            nc.vector.tensor_copy(out=w0b[:, c, :], in_=w0t[:, c, :])
            nc.vector.tensor_copy(out=wbb[:, c, :], in_=wbt[:, c, :])
            nc.vector.tensor_copy(out=w1b[:, c, :], in_=w1t[:, c, :])
        b0bc = consts.tile([P, D], f32)
        bbbc = consts.tile([P, D], f32)
        b1bc = consts.tile([P, D], f32)
        for h_, t_ in ((b0_h, b0bc), (bb_h, bbbc), (b1_h, b1bc)):
            nc.gpsimd.dma_start(
                out=t_[:], in_=bass.AP(tensor=h_, offset=0, ap=[[0, P], [1, D]])
            )
        g1r = consts.tile([1, D], f32)
        be1r = consts.tile([1, D], f32)
        g2r = consts.tile([1, D], f32)
        be2r = consts.tile([1, D], f32)
        nc.sync.dma_start(out=g1r[:], in_=g1_h[:])
        nc.sync.dma_start(out=be1r[:], in_=be1_h[:])
        nc.sync.dma_start(out=g2r[:], in_=g2_h[:])
        nc.sync.dma_start(out=be2r[:], in_=be2_h[:])
        ident = consts.tile([P, P], f32)
        make_identity(nc, ident[:])
        ident3 = consts.tile([P, 3, P], f32)
        for j3 in range(3):
            nc.vector.tensor_copy(out=ident3[:, j3, :], in_=ident[:])
        onesb = consts.tile([P, 1], bf16)
        nc.vector.memset(onesb[:], 1.0)
        epst = consts.tile([1, 1], f32)
        nc.vector.memset(epst[:], EPS)
        zerosf = consts.tile([P, D], f32)
        nc.vector.memset(zerosf[:], 0.0)

        # resident state (x2 reuses x1res: x1 is dead once H1' is written)
        degres = consts.tile([P, NTF], f32)       # reciprocal clamped tgt degree
        x1res = consts.tile([P, NTF, D], bf16)    # layer-1 pre-BN x / layer-2 x
        h1res = consts.tile([P, NTF, D], bf16)    # H1' (BN'd)
        x2res = x1res
        a1bc = consts.tile([P, D], f32)
        b1bc2 = consts.tile([P, D], f32)
        a2bc = consts.tile([P, D], f32)
        b2bc2 = consts.tile([P, D], f32)

        acc_pool = ctx.enter_context(tc.tile_pool(name="acc", bufs=4))
        ps_tr = ctx.enter_context(tc.tile_pool(name="pstr", bufs=2, space="PSUM"))
        ps_x = ctx.enter_context(tc.tile_pool(name="psx", bufs=1, space="PSUM"))
        ps_st = ctx.enter_context(tc.tile_pool(name="psst", bufs=1, space="PSUM"))
        zt_pool = ctx.enter_context(tc.tile_pool(name="zt", bufs=2))
        misc = ctx.enter_context(tc.tile_pool(name="misc", bufs=2))
        emb_pool = ctx.enter_context(tc.tile_pool(name="embp", bufs=2))
        d3_pool = ctx.enter_context(tc.tile_pool(name="d3p", bufs=2))

        st1 = ps_st.tile([1, 2 * D], f32, name="st1")
        st2 = ps_st.tile([1, 2 * D], f32, name="st2")
        st1x, st1q = st1[:, 0:D], st1[:, D : 2 * D]
        st2x, st2q = st2[:, 0:D], st2[:, D : 2 * D]

        qctr = [0]

        def agg_pass(ncols, idx_t, val_t, table_ap, gdt, g_pool, post):
            """Partition-aligned aggregation, flat cross-tile gather batches.

            Columns at batch slots 5..7 (when wholly inside one tile) are
            routed to the PE as diag(val) matmuls accumulating DIRECTLY into
            the transposed PSUM zT; the rest accumulate on the DVE
            (acc += gathered * val[p]) and are transposed into the same PSUM
            at tile end.  The degree scale is applied after the transform.
            """
            mlist = [int(x) for x in ncols]
            tcn = sum(mlist)
            bounds = []
            off = 0
            for m in mlist:
                bounds.append((off, off + m))
                off += m
            tile_of = np.repeat(np.arange(len(mlist)), mlist)
            acc = None
            psT = None
            pe_started = False
            b0 = 0
            while b0 < tcn:
                bs = min(GBT, tcn - b0)
                gt = g_pool.tile([P, GBT, D], gdt, tag="gt")
                nc.gpsimd.dma_gather(
                    out_ap=gt[:, 0:bs, :],
                    in_ap=table_ap,
                    idxs_ap=idx_t[:, b0 * 8 : (b0 + bs) * 8],
                    num_idxs=bs * P,
                    num_idxs_reg=bs * P,
                    elem_size=D,
                    single_packet=True,
                    queue_num=qctr[0] % NSWQ,
                )
                qctr[0] += 1
                # PE-offload batch slots 5..7 when they lie inside one tile
                pe_lo, pe_hi = b0 + 5, b0 + GBT - 1
                pe_ok = PE_OFFLOAD and bs == GBT and int(tile_of[pe_lo]) == int(tile_of[pe_hi])
                if pe_ok:
                    lt_pe = int(tile_of[pe_lo])
                    d3 = d3_pool.tile([P, 3, P], gdt, tag="d3")
                    vb = val_t[:, pe_lo : pe_lo + 3].to_broadcast([P, 3, P])
                    nc.vector.tensor_tensor(
                        out=d3[:], in0=vb, in1=ident3[:], op=AluOp.mult
                    )
                for i in range(bs):
                    col = b0 + i
                    lt = int(tile_of[col])
                    a0, a1_ = bounds[lt]
                    is_pe = pe_ok and pe_lo <= col <= pe_hi and col != a0
                    if is_pe:
                        if psT is None:
                            psT = ps_tr.tile([P, DC, P], f32, tag="psT")
                        for ch in range(DC):
                            nc.tensor.matmul(
                                out=psT[:, ch, :],
                                lhsT=gt[:, i, ch * P : (ch + 1) * P],
                                rhs=d3[:, col - pe_lo, :],
                                start=not pe_started,
                                stop=False,
                            )
                        pe_started = True
                    else:
                        nxt = acc_pool.tile([P, D], f32, tag="acc")
                        nc.vector.scalar_tensor_tensor(
                            out=nxt[:],
                            in0=gt[:, i, :],
                            scalar=val_t[:, col : col + 1],
                            in1=(zerosf[:] if col == a0 else acc[:]),
                            op0=AluOp.mult,
                            op1=AluOp.add,
                        )
                        acc = nxt
                    if col == a1_ - 1:
                        if psT is None:
                            psT = ps_tr.tile([P, DC, P], f32, tag="psT")
                        # fold the DVE accumulator in, transposed
                        for ch in range(DC):
                            nc.tensor.matmul(
                                out=psT[:, ch, :],
                                lhsT=acc[:, ch * P : (ch + 1) * P],
                                rhs=ident[:],
                                start=not pe_started,
                                stop=True,
                            )
                        dsum = misc.tile([P, 1], f32, tag="dsum")
                        nc.vector.tensor_reduce(
                            out=dsum[:], in_=val_t[:, a0:a1_], axis=Axis.X,
                            op=AluOp.add,
                        )
                        dcl = misc.tile([P, 1], f32, tag="dcl")
                        nc.vector.tensor_scalar_max(dcl[:], dsum[:], 1.0)
                        rd = misc.tile([P, 1], f32, tag="rd")
                        nc.vector.reciprocal(rd[:], dcl[:])
                        post(lt, psT, rd)
                        psT = None
                        pe_started = False
                b0 += bs

        def transform(psT, wchunks):
            """psT [P, DC, P] PSUM (feature-major agg) -> PSUM [P, D] = agg @ W."""
            zt = zt_pool.tile([P, DC, P], bf16, tag="zt")
            for c in range(DC):
                nc.scalar.copy(out=zt[:, c, :], in_=psT[:, c, :])
            ps = ps_x.tile([P, D], f32, tag="psx")
            nc.tensor.matmul(
                out=ps[:], lhsT=zt[:, 0, :], rhs=wchunks[:, 0, :],
                start=True, stop=False,
            )
            nc.tensor.matmul(
                out=ps[:], lhsT=zt[:, 1, :], rhs=wchunks[:, 1, :],
                start=False, stop=True,
            )
            return ps

        ones1f = consts.tile([1, P], f32)
        nc.vector.memset(ones1f[:], 1.0)
        ps_bn = ctx.enter_context(tc.tile_pool(name="psbn", bufs=1, space="PSUM"))

        def bn_ar(stx, stq, stin, stout):
            """Pack stats and launch the AllReduce."""
            st_sb = misc.tile([1, 2 * D], f32, tag="stsb")
            nc.scalar.copy(out=st_sb[:, 0:D], in_=stx)
            nc.scalar.copy(out=st_sb[:, D : 2 * D], in_=stq)
            nc.sync.dma_start(out=stin[:], in_=st_sb[:])
            return nc.gpsimd.collective_compute(
                "AllReduce", AluOp.add, replica_groups=rg,
                ins=[stin[:].opt()], outs=[stout[:].opt()],
            )

        def bn_cf(stout, g_r, be_r, abc, bbc):
            """Coefficients A=gamma/std, B=beta-mean*A, broadcast via PE."""
            stg = misc.tile([1, 2 * D], f32, tag="stg")
            nc.sync.dma_start(out=stg[:], in_=stout[:])
            mean = misc.tile([1, D], f32, tag="mean")
            nc.vector.tensor_scalar_mul(mean[:], stg[:, 0:D], 1.0 / N_TGT)
            q = misc.tile([1, D], f32, tag="q")
            nc.vector.tensor_scalar_mul(q[:], stg[:, D : 2 * D], 1.0 / N_TGT)
            musq = misc.tile([1, D], f32, tag="musq")
            nc.vector.tensor_mul(musq[:], mean[:], mean[:])
            var = misc.tile([1, D], f32, tag="var")
            nc.vector.tensor_tensor(
                out=var[:], in0=q[:], in1=musq[:], op=AluOp.subtract
            )
            sd = misc.tile([1, D], f32, tag="sd")
            nc.scalar.activation(out=sd[:], in_=var[:], func=Act.Sqrt,
                                 bias=epst[:])
            rstd = misc.tile([1, D], f32, tag="rstd")
            nc.vector.reciprocal(rstd[:], sd[:])
            ab = misc.tile([1, 2 * D], f32, tag="ab")
            nc.vector.tensor_mul(ab[:, 0:D], g_r[:], rstd[:])
            mA = misc.tile([1, D], f32, tag="mA")
            nc.vector.tensor_mul(mA[:], mean[:], ab[:, 0:D])
            nc.vector.tensor_tensor(
                out=ab[:, D : 2 * D], in0=be_r[:], in1=mA[:],
                op=AluOp.subtract,
            )
            psab = ps_bn.tile([P, 2 * D], f32, tag="psab")
            nc.tensor.matmul(out=psab[:], lhsT=ones1f[:], rhs=ab[:],
                             start=True, stop=True)
            nc.scalar.copy(out=abc[:], in_=psab[:, 0:D])
            nc.scalar.copy(out=bbc[:], in_=psab[:, D : 2 * D])

        # ================= pass B: layer-1 forward =====================
        def post_fwd0(lt, psT, rd):
            nc.vector.tensor_copy(out=degres[:, lt : lt + 1], in_=rd[:])
            ps = transform(psT, w0b)
            et = emb_pool.tile([P, D], f32, tag="emb")
            nc.sync.dma_start(out=et[:], in_=emb[lt * P : (lt + 1) * P, :])
            t1 = misc.tile([P, D], f32, tag="t1")
            nc.vector.scalar_tensor_tensor(
                out=t1[:], in0=ps[:], scalar=rd[:], in1=b0bc[:],
                op0=AluOp.mult, op1=AluOp.add,
            )
            t2 = misc.tile([P, D], f32, tag="t2")
            nc.scalar.activation(out=t2[:], in_=t1[:], func=Act.Relu)
            nc.vector.tensor_add(x1res[:, lt, :], t2[:], et[:])
            nc.sync.dma_start(
                out=X1_loc[lt * P : (lt + 1) * P, :], in_=x1res[:, lt, :]
            )
            sq = misc.tile([P, D], bf16, tag="sq")
            nc.vector.tensor_mul(sq[:], x1res[:, lt, :], x1res[:, lt, :])
            nc.tensor.matmul(
                out=st1x, lhsT=onesb[:], rhs=x1res[:, lt, :],
                start=(lt == 0), stop=(lt == NTF - 1),
            )
            nc.tensor.matmul(
                out=st1q, lhsT=onesb[:], rhs=sq[:],
                start=(lt == 0), stop=(lt == NTF - 1),
            )

        with tc.tile_pool(name="edgeE", bufs=1) as epe:
            bidx = epe.tile([P, TCB * 8], i16, name="bidx")
            bval = epe.tile([P, TCB], f32, name="bval")
            nc.scalar.dma_start(out=bidx[:], in_=be_i16[:])
            nc.scalar.dma_start(out=bval[:], in_=be_val[:])

            with tc.tile_pool(name="edgeB", bufs=1) as epb, \
                 tc.tile_pool(name="gB", bufs=5) as gB:
                fidx = epb.tile([P, TCF * 8], i16, name="fidx")
                fval = epb.tile([P, TCF], f32, name="fval")
                nc.sync.dma_start(out=fidx[:], in_=fe_i16[:])
                nc.sync.dma_start(out=fval[:], in_=fe_val[:])
                agg_pass(ncolsF, fidx, fval, Hsrc[N_SRC // 2 :, :], f32, gB,
                         post_fwd0)

            # -------- BN-1 (commuted): AllReduce, then AllGather raw x1;
            # coefficient work and H1' apply overlap the AllGather ---------
            bn_ar(st1x, st1q, st1_in, st1_out)
            # force AllReduce -> AllGather on the serial CC path: rewrite one
            # X1_loc element with an identical value (x + AR-dependent zero)
            tiny = misc.tile([1, 1], f32, tag="tiny")
            nc.sync.dma_start(out=tiny[:], in_=st1_out[0:1, 0:1])
            z0 = misc.tile([1, 1], f32, tag="z0")
            nc.vector.tensor_scalar_mul(z0[:], tiny[:], 0.0)
            x00 = misc.tile([1, 1], bf16, tag="x00")
            nc.vector.tensor_tensor(
                out=x00[:], in0=x1res[0:1, 0, 0:1], in1=z0[:], op=AluOp.add
            )
            nc.sync.dma_start(out=X1_loc[0:1, 0:1], in_=x00[:])
            nc.gpsimd.collective_compute(
                "AllGather", AluOp.bypass, replica_groups=rg,
                ins=[X1_loc[:].opt()], outs=[X1_full[:].opt()],
            )
            bn_cf(st1_out, g1r, be1r, a1bc, b1bc2)

            # f-major A1/B1 via PE transpose (first column of the transpose)
            a1f = consts.tile([P, DC], f32)
            b1f = consts.tile([P, DC], f32)
            for c in range(DC):
                pt = ps_tr.tile([P, P], f32, tag="tr")
                nc.tensor.transpose(out=pt[:], in_=a1bc[:, c * P : (c + 1) * P],
                                    identity=ident[:])
                nc.vector.tensor_copy(out=a1f[:, c : c + 1], in_=pt[:, 0:1])
                pt2 = ps_tr.tile([P, P], f32, tag="tr")
                nc.tensor.transpose(out=pt2[:], in_=b1bc2[:, c * P : (c + 1) * P],
                                    identity=ident[:])
                nc.vector.tensor_copy(out=b1f[:, c : c + 1], in_=pt2[:, 0:1])
            # WbA = diag(A1) @ Wb ; ebias = B1 @ Wb + bb (all per-feature)
            wbA = consts.tile([P, DC, D], bf16)
            for c in range(DC):
                nc.vector.scalar_tensor_tensor(
                    out=wbA[:, c, :], in0=wbt[:, c, :],
                    scalar=a1f[:, c : c + 1], in1=zerosf[:],
                    op0=AluOp.mult, op1=AluOp.add,
                )
            psbw = ps_x.tile([P, D], f32, tag="psx")
            nc.tensor.matmul(out=psbw[0:1, :], lhsT=b1f[:, 0:1],
                             rhs=wbt[:, 0, :], start=True, stop=False)
            nc.tensor.matmul(out=psbw[0:1, :], lhsT=b1f[:, 1:2],
                             rhs=wbt[:, 1, :], start=False, stop=True)
            bwr = misc.tile([1, D], f32, tag="bwr")
            nc.scalar.copy(out=bwr[:], in_=psbw[0:1, :])
            psbb = ps_x.tile([P, D], f32, tag="psx")
            nc.tensor.matmul(out=psbb[:], lhsT=ones1f[:], rhs=bwr[:],
                             start=True, stop=True)
            ebias = consts.tile([P, D], f32)
            nc.vector.tensor_add(ebias[:], psbb[:], bbbc[:])
            # H1' tiles for the layer-2 residual (overlaps the AllGather)
            for lt in range(NTF):
                tmp = misc.tile([P, D], f32, tag="h1tmp")
                nc.vector.tensor_mul(tmp[:], x1res[:, lt, :], a1bc[:])
                nc.vector.tensor_add(h1res[:, lt, :], tmp[:], b1bc2[:])

            # ================= pass E: layer-1 backward ================
            def post_bwd(lt, psT, rd):
                ps = transform(psT, wbA)
                t1 = misc.tile([P, D], f32, tag="t1")
                nc.vector.scalar_tensor_tensor(
                    out=t1[:], in0=ps[:], scalar=rd[:], in1=ebias[:],
                    op0=AluOp.mult, op1=AluOp.add,
                )
                hs = misc.tile([P, D], bf16, tag="hs")
                nc.scalar.activation(out=hs[:], in_=t1[:], func=Act.Relu)
                nc.sync.dma_start(
                    out=Hs1_loc[lt * P : (lt + 1) * P, :], in_=hs[:]
                )

            with tc.tile_pool(name="edgeG", bufs=1) as epg:
                gidx = epg.tile([P, TCF * 8], i16, name="gidx")
                gval = epg.tile([P, TCF], f32, name="gval")
                nc.scalar.dma_start(out=gidx[:], in_=ge_i16[:])
                nc.scalar.dma_start(out=gval[:], in_=ge_val[:])

                with tc.tile_pool(name="gE", bufs=8) as gE:
                    agg_pass(ncolsB, bidx, bval, X1_full[:], bf16, gE, post_bwd)

                nc.gpsimd.collective_compute(
                    "AllGather", AluOp.bypass, replica_groups=rg,
                    ins=[Hs1_loc[:].opt()], outs=[Hs1_full[:].opt()],
                )

                # ================= pass G: layer-2 forward =============
                def post_fwd1(lt, psT, rd):
                    ps = transform(psT, w1b)
                    t1 = misc.tile([P, D], f32, tag="t1")
                    nc.vector.scalar_tensor_tensor(
                        out=t1[:], in0=ps[:], scalar=degres[:, lt : lt + 1],
                        in1=b1bc[:], op0=AluOp.mult, op1=AluOp.add,
                    )
                    t2 = misc.tile([P, D], f32, tag="t2")
                    nc.scalar.activation(out=t2[:], in_=t1[:], func=Act.Relu)
                    nc.vector.tensor_add(
                        x2res[:, lt, :], t2[:], h1res[:, lt, :]
                    )
                    sq = misc.tile([P, D], bf16, tag="sq")
                    nc.vector.tensor_mul(
                        sq[:], x2res[:, lt, :], x2res[:, lt, :]
                    )
                    nc.tensor.matmul(
                        out=st2x, lhsT=onesb[:], rhs=x2res[:, lt, :],
                        start=(lt == 0), stop=(lt == NTF - 1),
                    )
                    nc.tensor.matmul(
                        out=st2q, lhsT=onesb[:], rhs=sq[:],
                        start=(lt == 0), stop=(lt == NTF - 1),
                    )

                with tc.tile_pool(name="gG", bufs=8) as gG:
                    agg_pass(ncolsF, gidx, gval, Hs1_full[N_SRC // 2 :, :],
                             bf16, gG, post_fwd1)

            # ---------------- BN-2 + output ----------------------------
            bn_ar(st2x, st2q, st2_in, st2_out)
            bn_cf(st2_out, g2r, be2r, a2bc, b2bc2)
            for lt in range(NTF):
                tmp = misc.tile([P, D], f32, tag="o1")
                nc.vector.tensor_mul(tmp[:], x2res[:, lt, :], a2bc[:])
                ot = misc.tile([P, D], f32, tag="ot")
                if lt % 2 == 0:
                    nc.vector.tensor_add(ot[:], tmp[:], b2bc2[:])
                else:
                    nc.gpsimd.tensor_tensor(
                        out=ot[:], in0=tmp[:], in1=b2bc2[:], op=AluOp.add
                    )
                eng = nc.sync if lt % 2 == 0 else nc.scalar
                eng.dma_start(
                    out=out_d[lt * P : (lt + 1) * P, :], in_=ot[:]
                )

    nc.compile()
    return nc


# ----------------------------------------------------------------- entry


def _run(inputs, trace=False, tmpdir=None):
    from concourse.bass_utils import run_bass_kernel_spmd

    H_src = np.asarray(inputs["H_src"], dtype=np.float32)
    target_emb = np.asarray(inputs["target_emb"], dtype=np.float32)
    W_fwd = np.asarray(inputs["W_fwd"], dtype=np.float32)
    b_fwd = np.asarray(inputs["b_fwd"], dtype=np.float32)
    W_bwd = np.asarray(inputs["W_bwd"], dtype=np.float32)
    b_bwd = np.asarray(inputs["b_bwd"], dtype=np.float32)
    gamma = np.asarray(inputs["gamma"], dtype=np.float32)
    beta = np.asarray(inputs["beta"], dtype=np.float32)
    vals = np.asarray(inputs["vals"], dtype=np.float32)
    rows = np.asarray(inputs["rows"]).astype(np.int64)
    cols = np.asarray(inputs["cols"]).astype(np.int64)

    n_src, D = H_src.shape
    n_tgt = target_emb.shape[0]
    assert D == D_FIXED and n_tgt == N_TGT and n_src == N_SRC

    fwd, bwd, idxB, valB, idxE, valE, idxG, valG = _make_plans(rows, cols, vals)

    ncolsF_core = fwd.ncols
    ncolsB_core = bwd.ncols
    nc = _build_program(list(ncolsF_core), list(ncolsB_core))

    # per-core permuted emb rows / output slots
    part = fwd.part  # [256, 128] target ids (post-swap)
    in_maps = []
    perms = []
    for c in range(NCORES):
        tiles = part[c::NCORES]              # [NTF, 128] lt-major
        perm = tiles.reshape(-1)
        perms.append(perm)
        in_maps.append(
            {
                "Hsrc": H_src,
                "emb": np.ascontiguousarray(target_emb[perm]),
                "W0": W_fwd[0],
                "Wb": W_bwd[0],
                "W1": W_fwd[1],
                "b0": b_fwd[0].reshape(1, D),
                "bb": b_bwd[0].reshape(1, D),
                "b1": b_fwd[1].reshape(1, D),
                "g1": gamma[0].reshape(1, D),
                "be1": beta[0].reshape(1, D),
                "g2": gamma[1].reshape(1, D),
                "be2": beta[1].reshape(1, D),
                "fe_i16": idxB[c],
                "fe_val": valB[c],
                "be_i16": idxE[c],
                "be_val": valE[c],
                "ge_i16": idxG[c],
                "ge_val": valG[c],
            }
        )

    res = run_bass_kernel_spmd(
        nc, in_maps, list(range(NCORES)), trace=trace, tmpdir=tmpdir
    )
    out = np.empty((N_TGT, D), np.float32)
    for c in range(NCORES):
        out[perms[c]] = np.asarray(res.results[c]["out"]).astype(np.float32)
    return out, res


def kernel(**inputs) -> np.ndarray:
    out, _ = _run(inputs)
    return out


# revision 29
# speedup vs baseline: 1.0350x; 1.0350x over previous
"""Bipartite GCN stack (2 layers) on 8 Trainium2 NeuronCores.

Architecture (v2): associativity + partition-aligned aggregation.

  - A @ (H W + b) == (A @ H) W + deg*b: every sparse aggregation runs on the
    RAW table (H_src / H1' / Hs1) and the dense d x d transform is applied
    per-destination afterwards.  No pre-transformed 64MB tables, no
    redundant dense work.
  - Aggregation: destinations are degree-sorted and dealt round-robin into
    128-row tiles (tile g -> core g%8), so every destination owns one SBUF
    partition.  Each gathered "column" holds one edge per destination;
    msg accumulation is a single DVE scalar_tensor_tensor per column
    (acc += gathered * val[p]), and the degree is a free-axis reduce of the
    val matrix.  No selection-matrix matmuls at all.
  - Gathers: dma_gather with 4 SWDGE queues round-robin (the Q7 ucode runs
    on core pair `queue_num`, so 4 queues pipeline ~2.4x).  65536-row
    tables are addressed with SIGNED int16 indices against a base biased by
    +32768 rows (the ucode sign-extends; only TRAILING negative indices are
    trimmed, so the planner guarantees the last slot of every call is
    non-negative via partition-127 edge placement).
  - BN stats via PE (ones^T @ [x | x^2] accumulated across tiles), 2KB
    AllReduce, coefficients broadcast via a DRAM round-trip.
  - Tables H1' (BN'd layer-1 targets) and Hs1 (layer-1 sources) are
    produced in slot order, AllGathered in bf16, and indexed through the
    host-side slot maps.

Host-side work: integer edge planning (sort/permute/pad) only; all FP math
runs on the NeuronCores.
"""

import numpy as np

P = 128
D_FIXED = 256
EPS = 1e-5
NCORES = 8
GBT = 8          # gather batch: columns (x128 rows) per dma_gather call
NSWQ = 4         # SWDGE queues used round-robin
PE_OFFLOAD = False

N_TGT = 32768
N_SRC = 65536


# ----------------------------------------------------------------- host plan


def _rank_within_group(sorted_groups):
    """Given a sorted array of group ids, return the rank of each element
    within its group (0,1,2,... per group)."""
    n = len(sorted_groups)
    if n == 0:
        return np.zeros(0, np.int64)
    first = np.r_[True, sorted_groups[1:] != sorted_groups[:-1]]
    starts = np.where(first, np.arange(n), 0)
    np.maximum.accumulate(starts, out=starts)
    return np.arange(n) - starts


def _constrained_positions(k, ncols, col0, tc):
    """Batch-final slot positions (<k) for a partition-127 destination with
    k edges in a tile spanning global columns [col0, col0+ncols) (batches cut
    at GLOBAL column multiples of 8, plus the very last column tc-1)."""
    cuts = [q - col0 for q in range(GBT - 1, col0 + ncols, GBT)
            if q >= col0]
    last = tc - 1 - col0
    if 0 <= last < ncols and last not in cuts:
        cuts.append(last)
    return [q for q in cuts if q < k]


class _SidePlan:
    pass


def _plan_side(dst, n_dst, ncores):
    """Degree-sorted partition-aligned destination layout for one direction.

    Returns a _SidePlan with:
      part:   [n_tiles, 128] destination ids per (global tile, partition)
      ncols:  [nlt] common column count per local tile
      slot:   [n_dst] -> (core*shard + lt*128 + p) table-row of each dst
      e_core/e_lt/e_p: per-edge placement (column assigned later per pass)
    """
    sp = _SidePlan()
    counts = np.bincount(dst, minlength=n_dst)
    order = np.argsort(-counts, kind="stable")
    n_tiles = n_dst // P
    nlt = n_tiles // ncores
    part = order.reshape(n_tiles, P).copy()
    band_max = counts[order].reshape(nlt, ncores * P).max(axis=1)
    ncols = np.maximum(band_max, 1).astype(np.int64)

    sp.counts = counts
    sp.part = part
    sp.ncols = ncols
    sp.n_tiles = n_tiles
    sp.nlt = nlt
    sp.n_dst = n_dst
    return sp


def _finish_side(sp, ncores, good_masks):
    """Pick partition-127 members (trailing-trim guard) and build slot maps.

    good_masks: list of [n_dst] bool arrays, one per biased pass using this
    side's layout (destination d needs >= |constrained| good edges for EVERY
    pass).  Empty list -> no constraint.
    """
    counts, part, ncols = sp.counts, sp.part, sp.ncols
    col_off = np.concatenate([[0], np.cumsum(ncols)])
    tc = int(col_off[-1])
    if good_masks:
        # per-destination good-edge counts per pass
        for g in range(sp.n_tiles):
            lt = g // ncores
            m = int(ncols[lt])
            c0 = int(col_off[lt])
            members = part[g]
            best, best_slack = 127, None
            for j in range(P):
                t = members[j]
                k = int(counts[t])
                ncon = len(_constrained_positions(k, m, c0, tc))
                slack = min(int(gm[t]) - ncon for gm in good_masks)
                if best_slack is None or slack > best_slack:
                    best, best_slack = j, slack
                    if slack >= 2:
                        break
            assert best_slack is not None and best_slack >= 0, (
                f"tile {g}: no viable partition-127 member (slack {best_slack})"
            )
            if best != 127:
                part[g, 127], part[g, best] = part[g, best], part[g, 127]

    slot = np.empty(sp.n_dst, np.int64)
    n_tiles = sp.n_tiles
    g_idx = np.arange(n_tiles)
    core_of_g = g_idx % ncores
    lt_of_g = g_idx // ncores
    shard = sp.n_dst // ncores
    base = core_of_g * shard + lt_of_g * P
    slot[part] = base[:, None] + np.arange(P)[None, :]
    sp.slot = slot
    sp.col_off = col_off
    sp.tc = tc
    return sp


def _assign_columns(sp, dst, tbl_idx, vals, ncores, constrain_good=None):
    """Assign each edge to (core, colg, p) and build idx/val arrays.

    tbl_idx: per-edge int16 table index (may be negative for biased tables).
    constrain_good: bool[n_edges] "good" mask; if given, partition-127
    destinations get good edges placed at constrained positions.
    Returns idx16 [ncores][128, TC*8], val [ncores][128, TC] f32.
    """
    slot = sp.slot[dst]
    shard = sp.n_dst // ncores
    core = slot // shard
    rem = slot % shard
    lt = rem // P
    p = rem % P

    # rank of each edge within its destination
    eorder = np.argsort(slot, kind="stable")
    rank = np.empty(len(dst), np.int64)
    rank[eorder] = _rank_within_group(slot[eorder])

    if constrain_good is not None:
        # re-rank edges of partition-127 destinations: good edges first at
        # constrained positions
        p127 = p == 127
        if p127.any():
            sub = np.where(p127)[0]
            sub_slot = slot[sub]
            so = np.argsort(sub_slot, kind="stable")
            sub_s = sub[so]
            ss = sub_slot[so]
            starts = np.r_[0, np.where(ss[1:] != ss[:-1])[0] + 1]
            ends = np.r_[starts[1:], len(ss)]
            for a, b in zip(starts, ends):
                es = sub_s[a:b]
                k = b - a
                lt_i = (ss[a] % shard) // P
                m = int(sp.ncols[lt_i])
                cons = _constrained_positions(
                    k, m, int(sp.col_off[lt_i]), sp.tc
                )
                good = constrain_good[es]
                order_pos = np.full(k, -1, np.int64)
                gi = np.where(good)[0]
                bi = np.where(~good)[0]
                assert len(gi) >= len(cons), "p127 guard violated"
                # good edges at constrained positions, rest fill remaining
                used = set()
                for q, e in zip(cons, gi):
                    order_pos[e] = q
                    used.add(q)
                rest = [q for q in range(k) if q not in used]
                pool = [e for e in gi[len(cons):]] + list(bi)
                for q, e in zip(rest, pool):
                    order_pos[e] = q
                rank[es] = order_pos

    colg = sp.col_off[lt] + rank
    tc = sp.tc

    idx16 = []
    valo = []
    for c in range(ncores):
        sel = core == c
        lin = np.zeros((tc, P), np.int16)
        va = np.zeros((tc, P), np.float32)
        lin[colg[sel], p[sel]] = tbl_idx[sel].astype(np.int16)
        va[colg[sel], p[sel]] = vals[sel]
        # idx layout: per GLOBAL batch [b0, b0+bs): [bs*128] -> [bs*8, 16].T
        out16 = np.zeros((16, tc * 8), np.int16)
        b0 = 0
        while b0 < tc:
            bs = min(GBT, tc - b0)
            blk = lin[b0 : b0 + bs].reshape(bs * P)
            out16[:, b0 * 8 : (b0 + bs) * 8] = blk.reshape(bs * 8, 16).T
            b0 += bs
        idx16.append(np.ascontiguousarray(np.tile(out16, (8, 1))))
        valo.append(np.ascontiguousarray(va.T))
    return idx16, valo


def _make_plans(rows, cols, vals):
    """Full host plan for all three aggregation passes."""
    fwd = _plan_side(rows, N_TGT, NCORES)
    bwd = _plan_side(cols, N_SRC, NCORES)

    # bwd has no trailing-trim constraint (table indices 0..32767 >= 0)
    _finish_side(bwd, NCORES, [])

    # fwd feeds pass B (idx = cols - 32768) and pass G (idx = hsrow - 32768)
    hsrow = bwd.slot  # source id -> Hs1 table row
    goodB = np.bincount(rows[cols >= N_SRC // 2], minlength=N_TGT)
    goodG = np.bincount(rows[hsrow[cols] >= N_SRC // 2], minlength=N_TGT)
    _finish_side(fwd, NCORES, [goodB, goodG])
    h1row = fwd.slot  # target id -> H1' table row

    idxB, valB = _assign_columns(
        fwd, rows, cols - N_SRC // 2, vals, NCORES,
        constrain_good=cols >= N_SRC // 2,
    )
    gmapped = hsrow[cols]
    idxG, valG = _assign_columns(
        fwd, rows, gmapped - N_SRC // 2, vals, NCORES,
        constrain_good=gmapped >= N_SRC // 2,
    )
    idxE, valE = _assign_columns(bwd, cols, h1row[rows], vals, NCORES)

    return fwd, bwd, idxB, valB, idxE, valE, idxG, valG


# ----------------------------------------------------------------- bass build


def _install_drain_patch():
    """walrus in this env allows only ONE sem-wait per instruction; split
    extra waits onto same-engine carrier instructions."""
    import concourse.mybir as mybir
    import concourse.tile as _tile
    from concourse.vector_clock import ScopedClock

    if getattr(_tile.TileContext, "_drain_split_patched", False):
        return

    def _split_drain_and_barrier(self, tick_clock, wait_clock):
        nc = self.nc
        drain_inst = nc.sync.drain()
        wait_clock.add_sem_waits(
            drain_inst.ins, ScopedClock({None: tick_clock.global_clock})
        )
        si = drain_inst.ins.sync_info
        waits = list(si.on_wait) if si and si.on_wait else []
        if len(waits) > 1:
            si.on_wait = waits[:1]
            drain_inst.ins.sync_info = si
            for i in range(1, len(waits)):
                extra = nc.sync.drain()
                esi = extra.ins.sync_info
                upd = list(esi.on_update) if esi and esi.on_update else []
                extra.ins.sync_info = mybir.SyncInfo(
                    on_wait=[waits[i]], on_update=upd
                )
        nc.all_engine_barrier()
        assert self.sems is not None
        popped = nc._tile_sem_poison_stack.pop()
        assert popped is self._sem_poison
        nc.clear_and_free_semaphores(list(self.sems.allocated().values()))
        nc.all_engine_barrier()

    _tile.TileContext._drain_and_barrier = _split_drain_and_barrier

    _orig_add = _tile.TileContext._add_instruction

    def _add_instruction_split(self, inst):
        si = inst.sync_info
        waits = list(si.on_wait) if si and si.on_wait else []
        if len(waits) > 1 and inst.engine != mybir.EngineType.Unassigned:
            for w in waits[:-1]:
                nop = mybir.InstNoOp(
                    name=self.nc.get_next_instruction_name(), ins=[], outs=[]
                )
                nop.engine = inst.engine
                nop.sync_info = mybir.SyncInfo(on_wait=[w], on_update=[])
                _orig_add(self, nop)
            si.on_wait = waits[-1:]
            inst.sync_info = si
        _orig_add(self, inst)

    _tile.TileContext._add_instruction = _add_instruction_split
    _tile.TileContext._drain_split_patched = True


def _build_program(ncolsF, ncolsB):
    from contextlib import ExitStack

    import concourse.bass as bass
    import concourse.mybir as mybir
    import concourse.tile as tile
    from concourse import bacc
    from concourse.masks import make_identity

    _install_drain_patch()

    dt = mybir.dt
    f32 = dt.float32
    bf16 = dt.bfloat16
    i16 = dt.int16
    D = D_FIXED
    DC = D // P
    NTF = len(ncolsF)   # fwd tiles per core (32)
    NTB = len(ncolsB)   # bwd tiles per core (64)
    TCF = int(sum(ncolsF))
    TCB = int(sum(ncolsB))
    tgt_sh = NTF * P
    src_sh = NTB * P
    AluOp = mybir.AluOpType
    Act = mybir.ActivationFunctionType
    Axis = mybir.AxisListType
    rg = [list(range(NCORES))]

    nc = bacc.Bacc(
        "TRN2", target_bir_lowering=False, debug=False,
        num_devices=NCORES, num_swdge_queues=NSWQ,
    )

    dram_t = nc.dram_tensor
    # gathered with signed idx against a base biased by +32768 rows
    Hsrc = dram_t("Hsrc", [N_SRC, D], f32, kind="ExternalInput").ap()
    emb = dram_t("emb", [tgt_sh, D], f32, kind="ExternalInput").ap()
    W0 = dram_t("W0", [D, D], f32, kind="ExternalInput").ap()
    Wb = dram_t("Wb", [D, D], f32, kind="ExternalInput").ap()
    W1 = dram_t("W1", [D, D], f32, kind="ExternalInput").ap()
    b0_h = dram_t("b0", [1, D], f32, kind="ExternalInput")
    bb_h = dram_t("bb", [1, D], f32, kind="ExternalInput")
    b1_h = dram_t("b1", [1, D], f32, kind="ExternalInput")
    g1_h = dram_t("g1", [1, D], f32, kind="ExternalInput").ap()
    be1_h = dram_t("be1", [1, D], f32, kind="ExternalInput").ap()
    g2_h = dram_t("g2", [1, D], f32, kind="ExternalInput").ap()
    be2_h = dram_t("be2", [1, D], f32, kind="ExternalInput").ap()
    fe_i16 = dram_t("fe_i16", [P, TCF * 8], i16, kind="ExternalInput").ap()
    fe_val = dram_t("fe_val", [P, TCF], f32, kind="ExternalInput").ap()
    be_i16 = dram_t("be_i16", [P, TCB * 8], i16, kind="ExternalInput").ap()
    be_val = dram_t("be_val", [P, TCB], f32, kind="ExternalInput").ap()
    ge_i16 = dram_t("ge_i16", [P, TCF * 8], i16, kind="ExternalInput").ap()
    ge_val = dram_t("ge_val", [P, TCF], f32, kind="ExternalInput").ap()
    out_d = dram_t("out", [tgt_sh, D], f32, kind="ExternalOutput").ap()

    with tile.TileContext(nc) as tc, ExitStack() as ctx:
        dram = ctx.enter_context(tc.tile_pool(name="dram", bufs=1, space="DRAM"))
        X1_loc = dram.tile([tgt_sh, D], bf16)
        X1_full = dram.tile([N_TGT, D], bf16, addr_space="Shared")
        Hs1_loc = dram.tile([src_sh, D], bf16)
        Hs1_full = dram.tile([N_SRC, D], bf16, addr_space="Shared")
        st1_in = dram.tile([1, 2 * D], f32)
        st1_out = dram.tile([1, 2 * D], f32, addr_space="Shared")
        st2_in = dram.tile([1, 2 * D], f32)
        st2_out = dram.tile([1, 2 * D], f32, addr_space="Shared")
        ab1_d = dram.tile([1, 2 * D], f32)
        ab2_d = dram.tile([1, 2 * D], f32)

        # ---------------- constants ------------------------------------
        consts = ctx.enter_context(tc.tile_pool(name="consts", bufs=1))
        w0t = consts.tile([P, DC, D], f32)
        wbt = consts.tile([P, DC, D], f32)
        w1t = consts.tile([P, DC, D], f32)
        for c in range(DC):
            nc.sync.dma_start(out=w0t[:, c, :], in_=W0[c * P : (c + 1) * P, :])
            nc.sync.dma_start(out=wbt[:, c, :], in_=Wb[c * P : (c + 1) * P, :])
            nc.sync.dma_start(out=w1t[:, c, :], in_=W1[c * P : (c + 1) * P, :])
        w0b = consts.tile([P, DC, D], bf16)
        wbb = consts.tile([P, DC, D], bf16)
        w1b = consts.tile([P, DC, D], bf16)
        for c in range(DC):
            nc.vector.tensor_copy(out=w0b[:, c, :], in_=w0t[:, c, :])
            nc.vector.tensor_copy(out=wbb[:, c, :], in_=wbt[:, c, :])
            nc.vector.tensor_copy(out=w1b[:, c, :], in_=w1t[:, c, :])
        b0bc = consts.tile([P, D], f32)
        bbbc = consts.tile([P, D], f32)
        b1bc = consts.tile([P, D], f32)
        for h_, t_ in ((b0_h, b0bc), (bb_h, bbbc), (b1_h, b1bc)):
            nc.gpsimd.dma_start(
                out=t_[:], in_=bass.AP(tensor=h_, offset=0, ap=[[0, P], [1, D]])
            )
        g1r = consts.tile([1, D], f32)
        be1r = consts.tile([1, D], f32)
        g2r = consts.tile([1, D], f32)
        be2r = consts.tile([1, D], f32)
        nc.sync.dma_start(out=g1r[:], in_=g1_h[:])
        nc.sync.dma_start(out=be1r[:], in_=be1_h[:])
        nc.sync.dma_start(out=g2r[:], in_=g2_h[:])
        nc.sync.dma_start(out=be2r[:], in_=be2_h[:])
        ident = consts.tile([P, P], f32)
        make_identity(nc, ident[:])
        onesb = consts.tile([P, 1], bf16)
        nc.vector.memset(onesb[:], 1.0)
        epst = consts.tile([1, 1], f32)
        nc.vector.memset(epst[:], EPS)
        zerosf = consts.tile([P, D], f32)
        nc.vector.memset(zerosf[:], 0.0)

        # resident state (x2 reuses x1res: x1 is dead once H1' is written)
        degres = consts.tile([P, NTF], f32)       # reciprocal clamped tgt degree
        x1res = consts.tile([P, NTF, D], bf16)    # layer-1 pre-BN x / layer-2 x
        h1res = consts.tile([P, NTF, D], bf16)    # H1' (BN'd)
        x2res = x1res
        a1bc = consts.tile([P, D], f32)
        b1bc2 = consts.tile([P, D], f32)
        a2bc = consts.tile([P, D], f32)
        b2bc2 = consts.tile([P, D], f32)

        acc_pool = ctx.enter_context(tc.tile_pool(name="acc", bufs=4))
        ps_tr = ctx.enter_context(tc.tile_pool(name="pstr", bufs=2, space="PSUM"))
        ps_x = ctx.enter_context(tc.tile_pool(name="psx", bufs=2, space="PSUM"))
        ps_st = ctx.enter_context(tc.tile_pool(name="psst", bufs=1, space="PSUM"))
        zt_pool = ctx.enter_context(tc.tile_pool(name="zt", bufs=2))
        misc = ctx.enter_context(tc.tile_pool(name="misc", bufs=2))
        emb_pool = ctx.enter_context(tc.tile_pool(name="embp", bufs=2))

        st1 = ps_st.tile([1, 2 * D], f32, name="st1")
        st2 = ps_st.tile([1, 2 * D], f32, name="st2")
        st1x, st1q = st1[:, 0:D], st1[:, D : 2 * D]
        st2x, st2q = st2[:, 0:D], st2[:, D : 2 * D]

        qctr = [0]

        def agg_pass(ncols, idx_t, val_t, table_ap, gdt, g_pool, post):
            """Partition-aligned aggregation, flat cross-tile gather batches:
            acc[p,:] += gathered[p,:] * val[p,col]; per-tile deg + post."""
            mlist = [int(x) for x in ncols]
            tcn = sum(mlist)
            bounds = []
            off = 0
            for m in mlist:
                bounds.append((off, off + m))
                off += m
            tile_of = np.repeat(np.arange(len(mlist)), mlist)
            acc = None
            b0 = 0
            while b0 < tcn:
                bs = min(GBT, tcn - b0)
                gt = g_pool.tile([P, GBT, D], gdt, tag="gt")
                nc.gpsimd.dma_gather(
                    out_ap=gt[:, 0:bs, :],
                    in_ap=table_ap,
                    idxs_ap=idx_t[:, b0 * 8 : (b0 + bs) * 8],
                    num_idxs=bs * P,
                    num_idxs_reg=bs * P,
                    elem_size=D,
                    single_packet=True,
                    queue_num=qctr[0] % NSWQ,
                )
                qctr[0] += 1
                for i in range(bs):
                    col = b0 + i
                    lt = int(tile_of[col])
                    a0, a1_ = bounds[lt]
                    nxt = acc_pool.tile([P, D], f32, tag="acc")
                    nc.vector.scalar_tensor_tensor(
                        out=nxt[:],
                        in0=gt[:, i, :],
                        scalar=val_t[:, col : col + 1],
                        in1=(zerosf[:] if col == a0 else acc[:]),
                        op0=AluOp.mult,
                        op1=AluOp.add,
                    )
                    acc = nxt
                    if col == a1_ - 1:
                        dsum = misc.tile([P, 1], f32, tag="dsum")
                        nc.vector.tensor_reduce(
                            out=dsum[:], in_=val_t[:, a0:a1_], axis=Axis.X,
                            op=AluOp.add,
                        )
                        dcl = misc.tile([P, 1], f32, tag="dcl")
                        nc.vector.tensor_scalar_max(dcl[:], dsum[:], 1.0)
                        rd = misc.tile([P, 1], f32, tag="rd")
                        nc.vector.reciprocal(rd[:], dcl[:])
                        post(lt, acc, rd)
                b0 += bs

        def transform(z, wchunks):
            """z [P, D] f32 (dst-major) -> PSUM [P, D] f32 = z @ W."""
            zt = zt_pool.tile([P, DC, P], bf16, tag="zt")
            for c in range(DC):
                pt = ps_tr.tile([P, P], f32, tag="tr")
                nc.tensor.transpose(
                    out=pt[:], in_=z[:, c * P : (c + 1) * P], identity=ident[:]
                )
                nc.scalar.copy(out=zt[:, c, :], in_=pt[:])
            ps = ps_x.tile([P, D], f32, tag="psx")
            nc.tensor.matmul(
                out=ps[:], lhsT=zt[:, 0, :], rhs=wchunks[:, 0, :],
                start=True, stop=False,
            )
            nc.tensor.matmul(
                out=ps[:], lhsT=zt[:, 1, :], rhs=wchunks[:, 1, :],
                start=False, stop=True,
            )
            return ps

        ones1f = consts.tile([1, P], f32)
        nc.vector.memset(ones1f[:], 1.0)
        ps_bn = ctx.enter_context(tc.tile_pool(name="psbn", bufs=1, space="PSUM"))

        def bn_ar(stx, stq, stin, stout):
            """Pack stats and launch the AllReduce."""
            st_sb = misc.tile([1, 2 * D], f32, tag="stsb")
            nc.scalar.copy(out=st_sb[:, 0:D], in_=stx)
            nc.scalar.copy(out=st_sb[:, D : 2 * D], in_=stq)
            nc.sync.dma_start(out=stin[:], in_=st_sb[:])
            return nc.gpsimd.collective_compute(
                "AllReduce", AluOp.add, replica_groups=rg,
                ins=[stin[:].opt()], outs=[stout[:].opt()],
            )

        def bn_cf(stout, g_r, be_r, abc, bbc):
            """Coefficients A=gamma/std, B=beta-mean*A, broadcast via PE."""
            stg = misc.tile([1, 2 * D], f32, tag="stg")
            nc.sync.dma_start(out=stg[:], in_=stout[:])
            mean = misc.tile([1, D], f32, tag="mean")
            nc.vector.tensor_scalar_mul(mean[:], stg[:, 0:D], 1.0 / N_TGT)
            q = misc.tile([1, D], f32, tag="q")
            nc.vector.tensor_scalar_mul(q[:], stg[:, D : 2 * D], 1.0 / N_TGT)
            musq = misc.tile([1, D], f32, tag="musq")
            nc.vector.tensor_mul(musq[:], mean[:], mean[:])
            var = misc.tile([1, D], f32, tag="var")
            nc.vector.tensor_tensor(
                out=var[:], in0=q[:], in1=musq[:], op=AluOp.subtract
            )
            sd = misc.tile([1, D], f32, tag="sd")
            nc.scalar.activation(out=sd[:], in_=var[:], func=Act.Sqrt,
                                 bias=epst[:])
            rstd = misc.tile([1, D], f32, tag="rstd")
            nc.vector.reciprocal(rstd[:], sd[:])
            ab = misc.tile([1, 2 * D], f32, tag="ab")
            nc.vector.tensor_mul(ab[:, 0:D], g_r[:], rstd[:])
            mA = misc.tile([1, D], f32, tag="mA")
            nc.vector.tensor_mul(mA[:], mean[:], ab[:, 0:D])
            nc.vector.tensor_tensor(
                out=ab[:, D : 2 * D], in0=be_r[:], in1=mA[:],
                op=AluOp.subtract,
            )
            psab = ps_bn.tile([P, 2 * D], f32, tag="psab")
            nc.tensor.matmul(out=psab[:], lhsT=ones1f[:], rhs=ab[:],
                             start=True, stop=True)
            nc.scalar.copy(out=abc[:], in_=psab[:, 0:D])
            nc.scalar.copy(out=bbc[:], in_=psab[:, D : 2 * D])

        # ================= pass B: layer-1 forward =====================
        def post_fwd0(lt, acc, rd):
            nc.vector.tensor_copy(out=degres[:, lt : lt + 1], in_=rd[:])
            z = misc.tile([P, D], f32, tag="z")
            nc.scalar.activation(
                out=z[:], in_=acc[:], func=Act.Copy, scale=rd[:]
            )
            ps = transform(z, w0b)
            et = emb_pool.tile([P, D], f32, tag="emb")
            nc.sync.dma_start(out=et[:], in_=emb[lt * P : (lt + 1) * P, :])
            t1 = misc.tile([P, D], f32, tag="t1")
            nc.vector.tensor_add(t1[:], ps[:], b0bc[:])
            t2 = misc.tile([P, D], f32, tag="t2")
            nc.scalar.activation(out=t2[:], in_=t1[:], func=Act.Relu)
            nc.vector.tensor_add(x1res[:, lt, :], t2[:], et[:])
            nc.sync.dma_start(
                out=X1_loc[lt * P : (lt + 1) * P, :], in_=x1res[:, lt, :]
            )
            sq = misc.tile([P, D], bf16, tag="sq")
            nc.vector.tensor_mul(sq[:], x1res[:, lt, :], x1res[:, lt, :])
            nc.tensor.matmul(
                out=st1x, lhsT=onesb[:], rhs=x1res[:, lt, :],
                start=(lt == 0), stop=(lt == NTF - 1),
            )
            nc.tensor.matmul(
                out=st1q, lhsT=onesb[:], rhs=sq[:],
                start=(lt == 0), stop=(lt == NTF - 1),
            )

        with tc.tile_pool(name="edgeE", bufs=1) as epe:
            bidx = epe.tile([P, TCB * 8], i16, name="bidx")
            bval = epe.tile([P, TCB], f32, name="bval")
            nc.scalar.dma_start(out=bidx[:], in_=be_i16[:])
            nc.scalar.dma_start(out=bval[:], in_=be_val[:])

            with tc.tile_pool(name="edgeB", bufs=1) as epb, \
                 tc.tile_pool(name="gB", bufs=5) as gB:
                fidx = epb.tile([P, TCF * 8], i16, name="fidx")
                fval = epb.tile([P, TCF], f32, name="fval")
                nc.sync.dma_start(out=fidx[:], in_=fe_i16[:])
                nc.sync.dma_start(out=fval[:], in_=fe_val[:])
                agg_pass(ncolsF, fidx, fval, Hsrc[N_SRC // 2 :, :], f32, gB,
                         post_fwd0)

            # -------- BN-1 (commuted): AllReduce, then AllGather raw x1;
            # coefficient work and H1' apply overlap the AllGather ---------
            bn_ar(st1x, st1q, st1_in, st1_out)
            # force AllReduce -> AllGather on the serial CC path: rewrite one
            # X1_loc element with an identical value (x + AR-dependent zero)
            tiny = misc.tile([1, 1], f32, tag="tiny")
            nc.sync.dma_start(out=tiny[:], in_=st1_out[0:1, 0:1])
            z0 = misc.tile([1, 1], f32, tag="z0")
            nc.vector.tensor_scalar_mul(z0[:], tiny[:], 0.0)
            x00 = misc.tile([1, 1], bf16, tag="x00")
            nc.vector.tensor_tensor(
                out=x00[:], in0=x1res[0:1, 0, 0:1], in1=z0[:], op=AluOp.add
            )
            nc.sync.dma_start(out=X1_loc[0:1, 0:1], in_=x00[:])
            nc.gpsimd.collective_compute(
                "AllGather", AluOp.bypass, replica_groups=rg,
                ins=[X1_loc[:].opt()], outs=[X1_full[:].opt()],
            )
            bn_cf(st1_out, g1r, be1r, a1bc, b1bc2)

            # f-major A1/B1 via PE transpose (first column of the transpose)
            a1f = consts.tile([P, DC], f32)
            b1f = consts.tile([P, DC], f32)
            for c in range(DC):
                pt = ps_tr.tile([P, P], f32, tag="tr")
                nc.tensor.transpose(out=pt[:], in_=a1bc[:, c * P : (c + 1) * P],
                                    identity=ident[:])
                nc.vector.tensor_copy(out=a1f[:, c : c + 1], in_=pt[:, 0:1])
                pt2 = ps_tr.tile([P, P], f32, tag="tr")
                nc.tensor.transpose(out=pt2[:], in_=b1bc2[:, c * P : (c + 1) * P],
                                    identity=ident[:])
                nc.vector.tensor_copy(out=b1f[:, c : c + 1], in_=pt2[:, 0:1])
            # WbA = diag(A1) @ Wb ; ebias = B1 @ Wb + bb (all per-feature)
            wbA = consts.tile([P, DC, D], bf16)
            for c in range(DC):
                nc.vector.scalar_tensor_tensor(
                    out=wbA[:, c, :], in0=wbt[:, c, :],
                    scalar=a1f[:, c : c + 1], in1=zerosf[:],
                    op0=AluOp.mult, op1=AluOp.add,
                )
            psbw = ps_x.tile([P, D], f32, tag="psx")
            nc.tensor.matmul(out=psbw[0:1, :], lhsT=b1f[:, 0:1],
                             rhs=wbt[:, 0, :], start=True, stop=False)
            nc.tensor.matmul(out=psbw[0:1, :], lhsT=b1f[:, 1:2],
                             rhs=wbt[:, 1, :], start=False, stop=True)
            bwr = misc.tile([1, D], f32, tag="bwr")
            nc.scalar.copy(out=bwr[:], in_=psbw[0:1, :])
            psbb = ps_x.tile([P, D], f32, tag="psx")
            nc.tensor.matmul(out=psbb[:], lhsT=ones1f[:], rhs=bwr[:],
                             start=True, stop=True)
            ebias = consts.tile([P, D], f32)
            nc.vector.tensor_add(ebias[:], psbb[:], bbbc[:])
            # H1' tiles for the layer-2 residual (overlaps the AllGather)
            for lt in range(NTF):
                tmp = misc.tile([P, D], f32, tag="h1tmp")
                nc.vector.tensor_mul(tmp[:], x1res[:, lt, :], a1bc[:])
                nc.vector.tensor_add(h1res[:, lt, :], tmp[:], b1bc2[:])

            # ================= pass E: layer-1 backward ================
            def post_bwd(lt, acc, rd):
                z = misc.tile([P, D], f32, tag="z")
                nc.scalar.activation(
                    out=z[:], in_=acc[:], func=Act.Copy, scale=rd[:]
                )
                ps = transform(z, wbA)
                t1 = misc.tile([P, D], f32, tag="t1")
                nc.vector.tensor_add(t1[:], ps[:], ebias[:])
                hs = misc.tile([P, D], bf16, tag="hs")
                nc.scalar.activation(out=hs[:], in_=t1[:], func=Act.Relu)
                nc.sync.dma_start(
                    out=Hs1_loc[lt * P : (lt + 1) * P, :], in_=hs[:]
                )

            with tc.tile_pool(name="edgeG", bufs=1) as epg:
                gidx = epg.tile([P, TCF * 8], i16, name="gidx")
                gval = epg.tile([P, TCF], f32, name="gval")
                nc.scalar.dma_start(out=gidx[:], in_=ge_i16[:])
                nc.scalar.dma_start(out=gval[:], in_=ge_val[:])

                with tc.tile_pool(name="gE", bufs=8) as gE:
                    agg_pass(ncolsB, bidx, bval, X1_full[:], bf16, gE, post_bwd)

                nc.gpsimd.collective_compute(
                    "AllGather", AluOp.bypass, replica_groups=rg,
                    ins=[Hs1_loc[:].opt()], outs=[Hs1_full[:].opt()],
                )

                # ================= pass G: layer-2 forward =============
                def post_fwd1(lt, acc, rd):
                    z = misc.tile([P, D], f32, tag="z")
                    nc.scalar.activation(
                        out=z[:], in_=acc[:], func=Act.Copy,
                        scale=degres[:, lt : lt + 1],
                    )
                    ps = transform(z, w1b)
                    t1 = misc.tile([P, D], f32, tag="t1")
                    nc.vector.tensor_add(t1[:], ps[:], b1bc[:])
                    t2 = misc.tile([P, D], f32, tag="t2")
                    nc.scalar.activation(out=t2[:], in_=t1[:], func=Act.Relu)
                    nc.vector.tensor_add(
                        x2res[:, lt, :], t2[:], h1res[:, lt, :]
                    )
                    sq = misc.tile([P, D], bf16, tag="sq")
                    nc.vector.tensor_mul(
                        sq[:], x2res[:, lt, :], x2res[:, lt, :]
                    )
                    nc.tensor.matmul(
                        out=st2x, lhsT=onesb[:], rhs=x2res[:, lt, :],
                        start=(lt == 0), stop=(lt == NTF - 1),
                    )
                    nc.tensor.matmul(
                        out=st2q, lhsT=onesb[:], rhs=sq[:],
                        start=(lt == 0), stop=(lt == NTF - 1),
                    )

                with tc.tile_pool(name="gG", bufs=8) as gG:
                    agg_pass(ncolsF, gidx, gval, Hs1_full[N_SRC // 2 :, :],
                             bf16, gG, post_fwd1)

            # ---------------- BN-2 + output ----------------------------
            bn_ar(st2x, st2q, st2_in, st2_out)
            bn_cf(st2_out, g2r, be2r, a2bc, b2bc2)
            for lt in range(NTF):
                tmp = misc.tile([P, D], f32, tag="o1")
                nc.vector.tensor_mul(tmp[:], x2res[:, lt, :], a2bc[:])
                ot = misc.tile([P, D], f32, tag="ot")
                if lt % 2 == 0:
                    nc.vector.tensor_add(ot[:], tmp[:], b2bc2[:])
                else:
                    nc.gpsimd.tensor_tensor(
                        out=ot[:], in0=tmp[:], in1=b2bc2[:], op=AluOp.add
                    )
                eng = nc.sync if lt % 2 == 0 else nc.scalar
                eng.dma_start(
                    out=out_d[lt * P : (lt + 1) * P, :], in_=ot[:]
                )

    nc.compile()
    return nc


# ----------------------------------------------------------------- entry


def _run(inputs, trace=False, tmpdir=None):
    from concourse.bass_utils import run_bass_kernel_spmd

    H_src = np.asarray(inputs["H_src"], dtype=np.float32)
    target_emb = np.asarray(inputs["target_emb"], dtype=np.float32)
    W_fwd = np.asarray(inputs["W_fwd"], dtype=np.float32)
    b_fwd = np.asarray(inputs["b_fwd"], dtype=np.float32)
    W_bwd = np.asarray(inputs["W_bwd"], dtype=np.float32)
    b_bwd = np.asarray(inputs["b_bwd"], dtype=np.float32)
    gamma = np.asarray(inputs["gamma"], dtype=np.float32)
    beta = np.asarray(inputs["beta"], dtype=np.float32)
    vals = np.asarray(inputs["vals"], dtype=np.float32)
    rows = np.asarray(inputs["rows"]).astype(np.int64)
    cols = np.asarray(inputs["cols"]).astype(np.int64)

    n_src, D = H_src.shape
    n_tgt = target_emb.shape[0]
    assert D == D_FIXED and n_tgt == N_TGT and n_src == N_SRC

    fwd, bwd, idxB, valB, idxE, valE, idxG, valG = _make_plans(rows, cols, vals)

    ncolsF_core = fwd.ncols
    ncolsB_core = bwd.ncols
    nc = _build_program(list(ncolsF_core), list(ncolsB_core))

    # per-core permuted emb rows / output slots
    part = fwd.part  # [256, 128] target ids (post-swap)
    in_maps = []
    perms = []
    for c in range(NCORES):
        tiles = part[c::NCORES]              # [NTF, 128] lt-major
        perm = tiles.reshape(-1)
        perms.append(perm)
        in_maps.append(
            {
                "Hsrc": H_src,
                "emb": np.ascontiguousarray(target_emb[perm]),
                "W0": W_fwd[0],
                "Wb": W_bwd[0],
                "W1": W_fwd[1],
                "b0": b_fwd[0].reshape(1, D),
                "bb": b_bwd[0].reshape(1, D),
                "b1": b_fwd[1].reshape(1, D),
                "g1": gamma[0].reshape(1, D),
                "be1": beta[0].reshape(1, D),
                "g2": gamma[1].reshape(1, D),
                "be2": beta[1].reshape(1, D),
                "fe_i16": idxB[c],
                "fe_val": valB[c],
                "be_i16": idxE[c],
                "be_val": valE[c],
                "ge_i16": idxG[c],
                "ge_val": valG[c],
            }
        )

    res = run_bass_kernel_spmd(
        nc, in_maps, list(range(NCORES)), trace=trace, tmpdir=tmpdir
    )
    out = np.empty((N_TGT, D), np.float32)
    for c in range(NCORES):
        out[perms[c]] = np.asarray(res.results[c]["out"]).astype(np.float32)
    return out, res


def kernel(**inputs) -> np.ndarray:
    out, _ = _run(inputs)
    return out
